# revision 15
# baseline (speedup 1.0000x reference)
"""Trainium2 Bass kernel for the MixedHighwayCell scan problem.

Reference computation (per timestep t, sequential over T=512):
    pre_t = h_{t-1} + alpha*wx_t + beta*(h_{t-1} @ W_h.T)
    h_t   = pre_t * rsqrt(mean(pre_t^2) + eps)
    out_t = h_t * silu(h_t)
with wx = x @ W.T + b precomputed for all t.

Strategy (8 cores, batch-sharded: 4 batch rows per core, no collectives):
  Phase A: uT[d, t, b] = alpha*(x @ W.T + b) computed in exact fp32,
           directly in transposed (d-major) layout, kept in SBUF.
  Phase B: sequential scan with the state kept UNNORMALIZED (pre_t) in
           d-major layout [128, 8, NB].  The rms scale s_t is pushed
           through the recurrent matmul:
               mm_t     = pre_{t-1} @ Wh_lp.T        (low precision OK:
                                                      result is scaled by beta)
               pre_t    = h_{t-1} + u_t + (beta*s_{t-1}) * mm_t
               h_{t-1}  = s_{t-1} * pre_{t-1}
           The matmul streams Wh^T (moving operand) against the thin
           stationary pre^T tiles; its b-major output is transposed back
           to d-major by PE-transposes whose "identity" operand is
           beta*s*I, fusing the scalar application for free.
"""

import math

import numpy as np

import concourse.bass as bass
import concourse.bacc as bacc
import concourse.tile as tile
from concourse import mybir
from concourse.bass_utils import run_bass_kernel_spmd

F32 = mybir.dt.float32
BF16 = mybir.dt.bfloat16
FP8 = mybir.dt.float8e4

P = 128          # partitions
D = 1024         # model dim
NDT = D // P     # d-tiles (8)
T_FULL = 512
B_FULL = 32
NC = 8           # cores
NB = B_FULL // NC  # batch rows per core (4)
EPS = 1e-6
ROWCHUNK = 512   # projection moving-chunk (rows of (t,b))


def build_nc(T_steps: int, alpha: float, beta: float, lp_dtype=BF16, wh_scale: float = 1.0):
    """Build the Bass program (identical SPMD program for all cores)."""
    nc = bacc.Bacc()

    xT = nc.declare_dram_parameter("xT", [D, T_steps, NB], F32, isOutput=False)
    WT = nc.declare_dram_parameter("WT", [D, D], F32, isOutput=False)
    WhT = nc.declare_dram_parameter("WhT", [D, D], lp_dtype, isOutput=False)
    h0T = nc.declare_dram_parameter("h0T", [D, NB], F32, isOutput=False)
    abias = nc.declare_dram_parameter("abias", [D, 1], F32, isOutput=False)
    h_out = nc.declare_dram_parameter("h_out", [T_steps + 1, P, NDT, NB], F32, isOutput=True)
    outs_out = nc.declare_dram_parameter("outs_out", [T_steps, P, NDT, NB], F32, isOutput=True)

    rows = T_steps * NB

    with tile.TileContext(nc) as tc:
        with (
            tc.tile_pool(name="slabs", bufs=1) as slabs,
            tc.tile_pool(name="projw", bufs=1) as projw,
            tc.tile_pool(name="projx", bufs=2) as projx,
            tc.tile_pool(name="projps", bufs=2, space="PSUM") as projps,
        ):
            # persistent SBUF slabs
            wh_slab = slabs.tile([P, NDT, D], lp_dtype)
            WhT_view = WhT.rearrange("(k p) e -> p k e", p=P)
            for kt in range(NDT):
                nc.sync.dma_start(out=wh_slab[:, kt, :], in_=WhT_view[:, kt, :])
            uT_slab = slabs.tile([P, T_steps, NDT, NB], F32)
            abias_sb = slabs.tile([P, NDT], F32)
            nc.sync.dma_start(out=abias_sb, in_=abias.rearrange("(k p) one -> p (k one)", p=P))

            # scan constants (init before the DMA-heavy projection so these
            # memsets don't inherit WAR waits against all 8 DMA queues)
            ones_col = slabs.tile([P, 1], F32)
            nc.vector.memset(ones_col, 1.0)
            ones_row = slabs.tile([1, P], F32)
            nc.vector.memset(ones_row, 1.0)
            one_1x1 = slabs.tile([1, 1], F32)
            nc.vector.memset(one_1x1, 1.0)
            eps_ap = slabs.tile([1, 1], F32)
            nc.vector.memset(eps_ap, EPS)
            prime_d = slabs.tile([P, 1], F32)
            nc.vector.tensor_copy(prime_d, abias_sb[:, 0:1])  # DVE observes abias DMA
            ident_lp = slabs.tile([NB, NB], BF16)
            nc.vector.memset(ident_lp, 0.0)
            nc.gpsimd.affine_select(
                out=ident_lp, in_=ident_lp,
                compare_op=mybir.AluOpType.not_equal,
                fill=1.0,
                base=0, pattern=[[-1, NB]], channel_multiplier=1,
            )

            # ---------------- Phase A: projection  uT = alpha*(x @ W.T + b) ----------------
            # projection SBUF pools stay open for the whole kernel: reusing
            # their DMA-written SBUF would attach WAR waits against all 8 DMA
            # queues to later instructions (walrus wait-count limit).
            if True:
                wt_slab = projw.tile([P, NDT, D], F32)
                WT_view = WT.rearrange("(k p) e -> p k e", p=P)
                for kt in range(NDT):
                    nc.sync.dma_start(out=wt_slab[:, kt, :], in_=WT_view[:, kt, :])

                xT_view = xT.rearrange("(k p) t b -> p k (t b)", p=P)
                off = 0
                while off < rows:
                    rc = min(ROWCHUNK, rows - off)
                    t0, nt = off // NB, rc // NB
                    xt = projx.tile([P, NDT, rc], F32, tag="xt")
                    for kt in range(NDT):
                        nc.sync.dma_start(out=xt[:, kt, :], in_=xT_view[:, kt, off:off + rc])
                    for et in range(NDT):
                        ps = projps.tile([P, rc], F32, tag="ps")
                        if off >= 2 * ROWCHUNK or et >= 2:
                            # dummy PE write into the slot: funnels the WAR
                            # edge (vs the DVE uT-copy that read this slot)
                            # into one PE instruction with a single wait
                            prev_t0 = max(0, t0 - (2 * ROWCHUNK) // NB) if et < 2 else t0
                            nc.tensor.transpose(
                                ps[0:NDT, 0:1],
                                uT_slab[0:1, prev_t0, :, 0],
                                one_1x1,
                            )
                        for kt in range(NDT):
                            nc.tensor.matmul(
                                ps,
                                wt_slab[:, kt, et * P:(et + 1) * P],
                                xt[:, kt, :],
                                start=(kt == 0),
                                stop=(kt == NDT - 1),
                            )
                        # uT[:, t0:t0+nt, et, :] = alpha*ps + abias[et]  (DVE:
                        # the ACT struct allows only one semaphore wait)
                        nc.vector.tensor_scalar(
                            out=uT_slab[:, t0:t0 + nt, et, :],
                            in0=ps.rearrange("p (t b) -> p t b", b=NB),
                            scalar1=float(alpha),
                            scalar2=abias_sb[:, et:et + 1],
                            op0=mybir.AluOpType.mult,
                            op1=mybir.AluOpType.add,
                        )
                    off += rc

            # ---------------- Phase B: the scan ----------------
            with (
                tc.tile_pool(name="state", bufs=2) as state,
                tc.tile_pool(name="work", bufs=2) as work,
                tc.tile_pool(name="mmps", bufs=1, space="PSUM") as mmps_pool,
                tc.tile_pool(name="mmtps", bufs=2, space="PSUM") as mmt_pool,
                tc.tile_pool(name="smallps", bufs=2, space="PSUM") as small_ps,
            ):
                use_fp8 = lp_dtype == FP8
                lp_cols = 16 if use_fp8 else NB
                # initial state: pre_0 = h0, s_0 = 1
                hT_prev = state.tile([P, NDT, NB], F32, tag="hT")
                nc.sync.dma_start(out=hT_prev, in_=h0T.rearrange("(k p) b -> p k b", p=P))
                pre_lp_prev = state.tile([P, NDT, lp_cols], lp_dtype, tag="prelp")
                nc.scalar.copy(pre_lp_prev[:, :, 0:NB], hT_prev)
                scol_prev = state.tile([NB, 1], F32, tag="scol")
                nc.scalar.activation(
                    scol_prev, ones_col[0:NB, :],
                    mybir.ActivationFunctionType.Copy, scale=1.0 / wh_scale,
                )  # s_0 = 1
                nc.sync.dma_start(out=h_out[0], in_=hT_prev)

                NH = D // 512  # moving chunks of the recurrent matmul (2)
                for k in range(1, T_steps + 1):
                    # (1) mm = pre_{k-1} @ Wh^T   [NB, D] in PSUM (b-major)
                    mm_ps = mmps_pool.tile([NB, D], F32, tag="mm")
                    if use_fp8:
                        # DoubleRow: contraction 256 per matmul over jt-pairs
                        for nch in range(NH):
                            for kt in range(NDT // 2):
                                nc.tensor.matmul(
                                    mm_ps[:, nch * 512:(nch + 1) * 512],
                                    pre_lp_prev[:, 2 * kt:2 * kt + 2, 0:NB],
                                    wh_slab[:, 2 * kt:2 * kt + 2, nch * 512:(nch + 1) * 512],
                                    start=(kt == 0),
                                    stop=(kt == NDT // 2 - 1),
                                    perf_mode=mybir.MatmulPerfMode.DoubleRow,
                                )
                    else:
                        for nch in range(NH):
                            for kt in range(NDT):
                                nc.tensor.matmul(
                                    mm_ps[:, nch * 512:(nch + 1) * 512],
                                    pre_lp_prev[:, kt, :],
                                    wh_slab[:, kt, nch * 512:(nch + 1) * 512],
                                    start=(kt == 0),
                                    stop=(kt == NDT - 1),
                                )
                    # (2) scale by s_{k-1}/wh_scale and cast to bf16 (DVE:
                    # keeps the ACT engine free of table switches)
                    mm_sb = work.tile([NB, NDT, P], BF16, tag="mmsb")
                    for nch in range(NH):
                        nc.vector.tensor_scalar_mul(
                            mm_sb[:, nch * 4:(nch + 1) * 4, :],
                            mm_ps[:, nch * 512:(nch + 1) * 512].rearrange(
                                "p (a c) -> p a c", c=P),
                            scol_prev,
                        )
                    # (3) transpose back to d-major, scaled by beta*s_{k-1}
                    mmT_ps = mmt_pool.tile([P, NDT, NB], BF16, tag="mmt")
                    for jt in range(NDT):
                        nc.tensor.transpose(mmT_ps[:, jt, :], mm_sb[:, jt, :], ident_lp)
                    # (4) pre_k = h_{k-1} + u_k + mmT
                    tmp = work.tile([P, NDT, NB], F32, tag="tmp")
                    nc.vector.tensor_add(tmp, hT_prev, uT_slab[:, k - 1, :, :])
                    preT = work.tile([P, NDT, NB], F32, tag="pre")
                    nc.vector.tensor_add(preT, tmp, mmT_ps)
                    # (5) cast for the next matmul (critical path!)
                    pre_lp = state.tile([P, NDT, lp_cols], lp_dtype, tag="prelp")
                    nc.vector.tensor_copy(pre_lp[:, :, 0:NB], preT)

                    # ---- norm branch (off critical path) ----
                    sq = work.tile([P, NDT, NB], F32, tag="sq")
                    nc.vector.tensor_mul(sq, preT, preT)
                    ssq_ps = small_ps.tile([1, NDT, NB], F32, tag="small")
                    nc.tensor.matmul(
                        ssq_ps.rearrange("p a b -> p (a b)"),
                        ones_col,
                        sq.rearrange("p a b -> p (a b)"),
                        start=True, stop=True,
                    )
                    ssq_r = work.tile([1, NB], F32, tag="ssqr")
                    nc.vector.tensor_reduce(
                        ssq_r, ssq_ps.rearrange("p a b -> p b a"),
                        axis=mybir.AxisListType.X, op=mybir.AluOpType.add,
                    )
                    s_tmp = work.tile([1, NB], F32, tag="stmp")
                    nc.scalar.activation(
                        s_tmp, ssq_r, mybir.ActivationFunctionType.Sqrt,
                        bias=eps_ap, scale=1.0 / D,
                    )
                    s_row = work.tile([1, NB], F32, tag="srow")
                    nc.vector.reciprocal(s_row, s_tmp)
                    # replicate s_row 8x along free for the broadcast matmul
                    srep = work.tile([1, NDT, NB], F32, tag="srep")
                    nc.vector.tensor_copy(
                        out=srep,
                        in_=bass.AP(
                            tensor=s_row.tensor, offset=s_row.offset,
                            ap=[s_row.ap[0], [0, NDT], s_row.ap[-1]],
                        ),
                    )
                    # s as a column (for the diag build)
                    scol_ps = small_ps.tile([NB, 1], F32, tag="small")
                    nc.tensor.transpose(scol_ps, s_row, one_1x1)
                    scol = state.tile([NB, 1], F32, tag="scol")
                    nc.vector.tensor_scalar_mul(scol, scol_ps, 1.0 / wh_scale)
                    # broadcast s across partitions: sbc = ones_col128 x srep
                    sbc_ps = small_ps.tile([P, NDT, NB], F32, tag="small")
                    nc.tensor.matmul(
                        sbc_ps.rearrange("p a b -> p (a b)"),
                        ones_row,
                        srep.rearrange("p a b -> p (a b)"),
                        start=True, stop=True,
                    )
                    # h_k = s_k * pre_k, written into the (consumed) u slot
                    # k-1 of the slab: phase C batches the silu outputs later
                    hT = uT_slab[:, k - 1, :, :]
                    nc.vector.tensor_mul(hT, preT, sbc_ps)
                    nc.sync.dma_start(out=h_out[k], in_=hT)

                    hT_prev, pre_lp_prev, scol_prev = hT, pre_lp, scol

            # ---------------- Phase C: outs = h * silu(h), batched ----------------
            with (
                tc.tile_pool(name="phc", bufs=3) as phc,
            ):
                TCH = min(32, T_steps)  # timesteps per chunk
                for c0 in range(0, T_steps, TCH):
                    hch = uT_slab[:, c0:c0 + TCH, :, :]
                    sg = phc.tile([P, TCH, NDT, NB], F32, tag="sg")
                    nc.scalar.activation(
                        sg, hch, mybir.ActivationFunctionType.Sigmoid)
                    hsg = phc.tile([P, TCH, NDT, NB], F32, tag="hsg")
                    nc.vector.tensor_mul(hsg, hch, sg)
                    outc = phc.tile([P, TCH, NDT, NB], F32, tag="outc")
                    nc.vector.tensor_mul(outc, hsg, hch)
                    nc.sync.dma_start(
                        out=outs_out.rearrange("t p k b -> p t k b")[:, c0:c0 + TCH, :, :],
                        in_=outc)

    nc.compile()  # bacc legalization: ≤1 sync wait per instruction
    return nc


_NC_CACHE: dict = {}


def _get_nc(key, *args, **kwargs):
    if key not in _NC_CACHE:
        _NC_CACHE[key] = build_nc(*args, **kwargs)
    return _NC_CACHE[key]


def prepare_in_maps(x, h0, W, W_h, b, alpha, beta, lp_np=None, wh_scale: float = 1.0):
    """Host-side shard + transpose prep. Returns list of per-core input dicts."""
    import ml_dtypes
    if lp_np is None:
        lp_np = ml_dtypes.bfloat16
    T_steps = x.shape[0]
    WT_h = np.ascontiguousarray(W.T.astype(np.float32))
    WhT_h = np.ascontiguousarray((W_h.T * (beta * wh_scale)).astype(lp_np))
    abias_h = np.ascontiguousarray((alpha * b).astype(np.float32).reshape(D, 1))
    in_maps = []
    for c in range(NC):
        sl = slice(c * NB, (c + 1) * NB)
        xT_h = np.ascontiguousarray(x[:, sl, :].transpose(2, 0, 1).astype(np.float32))
        h0T_h = np.ascontiguousarray(h0[sl].T.astype(np.float32))
        in_maps.append({
            "xT": xT_h, "WT": WT_h, "WhT": WhT_h, "h0T": h0T_h, "abias": abias_h,
        })
    return in_maps


def assemble(results, T_steps):
    outs = np.empty((T_steps, B_FULL, D), np.float32)
    h = np.empty((T_steps + 1, B_FULL, D), np.float32)
    for c, r in enumerate(results):
        sl = slice(c * NB, (c + 1) * NB)
        outs[:, sl, :] = r["outs_out"].transpose(0, 3, 2, 1).reshape(T_steps, NB, D)
        h[:, sl, :] = r["h_out"].transpose(0, 3, 2, 1).reshape(T_steps + 1, NB, D)
    return outs, h


def run(x, h0, W, W_h, b, log_alpha, log_beta, trace=False, use_fp8=True):
    x = np.asarray(x, np.float32)
    h0 = np.asarray(h0, np.float32)
    W = np.asarray(W, np.float32)
    W_h = np.asarray(W_h, np.float32)
    b = np.asarray(b, np.float32)
    alpha = float(np.exp(np.float32(log_alpha)))
    beta = float(1.0 / (1.0 + math.exp(-float(log_beta))) * 0.1)
    T_steps = x.shape[0]

    import ml_dtypes
    if use_fp8:
        nc = _get_nc(("fp8", T_steps, alpha, beta), T_steps, alpha, beta,
                     lp_dtype=FP8, wh_scale=4096.0)
        in_maps = prepare_in_maps(x, h0, W, W_h, b, alpha, beta,
                                  lp_np=ml_dtypes.float8_e4m3, wh_scale=4096.0)
    else:
        nc = _get_nc(("v1", T_steps, alpha, beta), T_steps, alpha, beta)
        in_maps = prepare_in_maps(x, h0, W, W_h, b, alpha, beta)
    res = run_bass_kernel_spmd(nc, in_maps, list(range(NC)), trace=trace)
    outs, h = assemble(res.results, T_steps)
    return outs, h, res


def kernel(x, h0, W, W_h, b, log_alpha, log_beta):
    outs, h, _ = run(x, h0, W, W_h, b, log_alpha, log_beta)
    return outs, h


# revision 16
# speedup vs baseline: 1.5347x; 1.5347x over previous
"""Trainium2 Bass kernel for the MixedHighwayCell scan problem.

Reference computation (per timestep t, sequential over T=512):
    pre_t = h_{t-1} + alpha*wx_t + beta*(h_{t-1} @ W_h.T)
    h_t   = pre_t * rsqrt(mean(pre_t^2) + eps)
    out_t = h_t * silu(h_t)
with wx = x @ W.T + b precomputed for all t.

Strategy (8 cores, batch-sharded: 4 batch rows per core, no collectives):
  Phase A: uT[d, t, b] = alpha*(x @ W.T + b) computed in exact fp32,
           directly in transposed (d-major) layout, kept in SBUF.
  Phase B: sequential scan with the state kept UNNORMALIZED (pre_t) in
           d-major layout [128, 8, NB].  The rms scale s_t is pushed
           through the recurrent matmul:
               mm_t     = pre_{t-1} @ Wh_lp.T        (low precision OK:
                                                      result is scaled by beta)
               pre_t    = h_{t-1} + u_t + (beta*s_{t-1}) * mm_t
               h_{t-1}  = s_{t-1} * pre_{t-1}
           The matmul streams Wh^T (moving operand) against the thin
           stationary pre^T tiles; its b-major output is transposed back
           to d-major by PE-transposes whose "identity" operand is
           beta*s*I, fusing the scalar application for free.
"""

import math

import numpy as np

import concourse.bass as bass
import concourse.bacc as bacc
import concourse.tile as tile
from concourse import mybir
from concourse.bass_utils import run_bass_kernel_spmd

F32 = mybir.dt.float32
BF16 = mybir.dt.bfloat16
FP8 = mybir.dt.float8e4

P = 128          # partitions
D = 1024         # model dim
NDT = D // P     # d-tiles (8)
T_FULL = 512
B_FULL = 32
NC = 8           # cores
NB = B_FULL // NC  # batch rows per core (4)
EPS = 1e-6
ROWCHUNK = 512   # projection moving-chunk (rows of (t,b))


def build_nc(T_steps: int, alpha: float, beta: float, lp_dtype=BF16, wh_scale: float = 1.0):
    """Build the Bass program (identical SPMD program for all cores)."""
    nc = bacc.Bacc()

    xT = nc.declare_dram_parameter("xT", [D, T_steps, NB], F32, isOutput=False)
    WT = nc.declare_dram_parameter("WT", [D, D], F32, isOutput=False)
    WhT = nc.declare_dram_parameter("WhT", [D, D], lp_dtype, isOutput=False)
    h0T = nc.declare_dram_parameter("h0T", [D, NB], F32, isOutput=False)
    abias = nc.declare_dram_parameter("abias", [D, 1], F32, isOutput=False)
    h_out = nc.declare_dram_parameter("h_out", [T_steps + 1, P, NDT, NB], F32, isOutput=True)
    outs_out = nc.declare_dram_parameter("outs_out", [T_steps, P, NDT, NB], F32, isOutput=True)

    rows = T_steps * NB

    with tile.TileContext(nc) as tc:
        with (
            tc.tile_pool(name="slabs", bufs=1) as slabs,
            tc.tile_pool(name="projw", bufs=1) as projw,
            tc.tile_pool(name="projx", bufs=2) as projx,
            tc.tile_pool(name="projps", bufs=2, space="PSUM") as projps,
        ):
            # persistent SBUF slabs
            wh_slab = slabs.tile([P, NDT, D], lp_dtype)
            WhT_view = WhT.rearrange("(k p) e -> p k e", p=P)
            for kt in range(NDT):
                nc.sync.dma_start(out=wh_slab[:, kt, :], in_=WhT_view[:, kt, :])
            uT_slab = slabs.tile([P, T_steps, NDT, NB], F32)
            abias_sb = slabs.tile([P, NDT], F32)
            nc.sync.dma_start(out=abias_sb, in_=abias.rearrange("(k p) one -> p (k one)", p=P))

            # scan constants (init before the DMA-heavy projection so these
            # memsets don't inherit WAR waits against all 8 DMA queues)
            ones_col = slabs.tile([P, 1], F32)
            nc.vector.memset(ones_col, 1.0)
            ones_row = slabs.tile([1, P], F32)
            nc.vector.memset(ones_row, 1.0)
            one_1x1 = slabs.tile([1, 1], F32)
            nc.vector.memset(one_1x1, 1.0)
            eps_ap = slabs.tile([1, 1], F32)
            nc.vector.memset(eps_ap, EPS)
            prime_d = slabs.tile([P, 1], F32)
            nc.vector.tensor_copy(prime_d, abias_sb[:, 0:1])  # DVE observes abias DMA
            ident_lp = slabs.tile([NB, NB], BF16)
            nc.vector.memset(ident_lp, 0.0)
            nc.gpsimd.affine_select(
                out=ident_lp, in_=ident_lp,
                compare_op=mybir.AluOpType.not_equal,
                fill=1.0,
                base=0, pattern=[[-1, NB]], channel_multiplier=1,
            )

            # ---------------- Phase A: projection  uT = alpha*(x @ W.T + b) ----------------
            # projection SBUF pools stay open for the whole kernel: reusing
            # their DMA-written SBUF would attach WAR waits against all 8 DMA
            # queues to later instructions (walrus wait-count limit).
            if True:
                wt_slab = projw.tile([P, NDT, D], F32)
                WT_view = WT.rearrange("(k p) e -> p k e", p=P)
                for kt in range(NDT):
                    nc.sync.dma_start(out=wt_slab[:, kt, :], in_=WT_view[:, kt, :])

                xT_view = xT.rearrange("(k p) t b -> p k (t b)", p=P)
                off = 0
                while off < rows:
                    rc = min(ROWCHUNK, rows - off)
                    t0, nt = off // NB, rc // NB
                    xt = projx.tile([P, NDT, rc], F32, tag="xt")
                    for kt in range(NDT):
                        nc.sync.dma_start(out=xt[:, kt, :], in_=xT_view[:, kt, off:off + rc])
                    for et in range(NDT):
                        ps = projps.tile([P, rc], F32, tag="ps")
                        if off >= 2 * ROWCHUNK or et >= 2:
                            # dummy PE write into the slot: funnels the WAR
                            # edge (vs the DVE uT-copy that read this slot)
                            # into one PE instruction with a single wait
                            prev_t0 = max(0, t0 - (2 * ROWCHUNK) // NB) if et < 2 else t0
                            nc.tensor.transpose(
                                ps[0:NDT, 0:1],
                                uT_slab[0:1, prev_t0, :, 0],
                                one_1x1,
                            )
                        for kt in range(NDT):
                            nc.tensor.matmul(
                                ps,
                                wt_slab[:, kt, et * P:(et + 1) * P],
                                xt[:, kt, :],
                                start=(kt == 0),
                                stop=(kt == NDT - 1),
                            )
                        # uT[:, t0:t0+nt, et, :] = alpha*ps + abias[et]  (DVE:
                        # the ACT struct allows only one semaphore wait)
                        nc.vector.tensor_scalar(
                            out=uT_slab[:, t0:t0 + nt, et, :],
                            in0=ps.rearrange("p (t b) -> p t b", b=NB),
                            scalar1=float(alpha),
                            scalar2=abias_sb[:, et:et + 1],
                            op0=mybir.AluOpType.mult,
                            op1=mybir.AluOpType.add,
                        )
                    off += rc

            # ---------------- Phase B: the scan ----------------
            with (
                tc.tile_pool(name="state", bufs=2) as state,
                tc.tile_pool(name="work", bufs=2) as work,
                tc.tile_pool(name="mmps", bufs=1, space="PSUM") as mmps_pool,
                tc.tile_pool(name="mmtps", bufs=2, space="PSUM") as mmt_pool,
                tc.tile_pool(name="smallps", bufs=2, space="PSUM") as small_ps,
            ):
                use_fp8 = lp_dtype == FP8
                lp_cols = 16 if use_fp8 else NB
                # initial state: pre_0 = h0, s_0 = 1
                hT_prev = state.tile([P, NDT, NB], F32, tag="hT")
                nc.sync.dma_start(out=hT_prev, in_=h0T.rearrange("(k p) b -> p k b", p=P))
                pre_lp_prev = state.tile([P, NDT, lp_cols], lp_dtype, tag="prelp")
                nc.scalar.copy(pre_lp_prev[:, :, 0:NB], hT_prev)
                scol_prev = state.tile([NB, 1], F32, tag="scol")
                nc.scalar.activation(
                    scol_prev, ones_col[0:NB, :],
                    mybir.ActivationFunctionType.Copy, scale=1.0 / wh_scale,
                )  # s_0 = 1
                nc.sync.dma_start(out=h_out[0], in_=hT_prev)

                NH = D // 512  # moving chunks of the recurrent matmul (2)
                for k in range(1, T_steps + 1):
                    # (1) mm = pre_{k-1} @ Wh^T   [NB, D] in PSUM (b-major)
                    mm_ps = mmps_pool.tile([NB, D], F32, tag="mm")
                    if use_fp8:
                        # DoubleRow: contraction 256 per matmul over jt-pairs
                        for nch in range(NH):
                            for kt in range(NDT // 2):
                                nc.tensor.matmul(
                                    mm_ps[:, nch * 512:(nch + 1) * 512],
                                    pre_lp_prev[:, 2 * kt:2 * kt + 2, 0:NB],
                                    wh_slab[:, 2 * kt:2 * kt + 2, nch * 512:(nch + 1) * 512],
                                    start=(kt == 0),
                                    stop=(kt == NDT // 2 - 1),
                                    perf_mode=mybir.MatmulPerfMode.DoubleRow,
                                )
                    else:
                        for nch in range(NH):
                            for kt in range(NDT):
                                nc.tensor.matmul(
                                    mm_ps[:, nch * 512:(nch + 1) * 512],
                                    pre_lp_prev[:, kt, :],
                                    wh_slab[:, kt, nch * 512:(nch + 1) * 512],
                                    start=(kt == 0),
                                    stop=(kt == NDT - 1),
                                )
                    # (2) scale by s_{k-1}/wh_scale and cast to bf16 (DVE:
                    # keeps the ACT engine free of table switches)
                    mm_sb = work.tile([NB, NDT, P], BF16, tag="mmsb")
                    for nch in range(NH):
                        nc.vector.tensor_scalar_mul(
                            mm_sb[:, nch * 4:(nch + 1) * 4, :],
                            mm_ps[:, nch * 512:(nch + 1) * 512].rearrange(
                                "p (a c) -> p a c", c=P),
                            scol_prev,
                        )
                    # (3) transpose back to d-major, scaled by beta*s_{k-1}
                    mmT_ps = mmt_pool.tile([P, NDT, NB], BF16, tag="mmt")
                    for jt in range(NDT):
                        nc.tensor.transpose(mmT_ps[:, jt, :], mm_sb[:, jt, :], ident_lp)
                    # (4) pre_k = h_{k-1} + u_k + mmT
                    tmp = work.tile([P, NDT, NB], F32, tag="tmp")
                    nc.vector.tensor_add(tmp, hT_prev, uT_slab[:, k - 1, :, :])
                    preT = work.tile([P, NDT, NB], F32, tag="pre")
                    nc.vector.tensor_add(preT, tmp, mmT_ps)
                    # (5) cast for the next matmul (critical path!)
                    pre_lp = state.tile([P, NDT, lp_cols], lp_dtype, tag="prelp")
                    nc.vector.tensor_copy(pre_lp[:, :, 0:NB], preT)

                    # ---- norm branch (off critical path) ----
                    sq = work.tile([P, NDT, NB], F32, tag="sq")
                    nc.vector.tensor_mul(sq, preT, preT)
                    ssq_ps = small_ps.tile([1, NDT, NB], F32, tag="small")
                    nc.tensor.matmul(
                        ssq_ps.rearrange("p a b -> p (a b)"),
                        ones_col,
                        sq.rearrange("p a b -> p (a b)"),
                        start=True, stop=True,
                    )
                    ssq_r = work.tile([1, NB], F32, tag="ssqr")
                    nc.vector.tensor_reduce(
                        ssq_r, ssq_ps.rearrange("p a b -> p b a"),
                        axis=mybir.AxisListType.X, op=mybir.AluOpType.add,
                    )
                    s_tmp = work.tile([1, NB], F32, tag="stmp")
                    nc.scalar.activation(
                        s_tmp, ssq_r, mybir.ActivationFunctionType.Sqrt,
                        bias=eps_ap, scale=1.0 / D,
                    )
                    s_row = work.tile([1, NB], F32, tag="srow")
                    nc.vector.reciprocal(s_row, s_tmp)
                    # replicate s_row 8x along free for the broadcast matmul
                    srep = work.tile([1, NDT, NB], F32, tag="srep")
                    nc.vector.tensor_copy(
                        out=srep,
                        in_=bass.AP(
                            tensor=s_row.tensor, offset=s_row.offset,
                            ap=[s_row.ap[0], [0, NDT], s_row.ap[-1]],
                        ),
                    )
                    # s as a column (for the diag build)
                    scol_ps = small_ps.tile([NB, 1], F32, tag="small")
                    nc.tensor.transpose(scol_ps, s_row, one_1x1)
                    scol = state.tile([NB, 1], F32, tag="scol")
                    nc.vector.tensor_scalar_mul(scol, scol_ps, 1.0 / wh_scale)
                    # broadcast s across partitions: sbc = ones_col128 x srep
                    sbc_ps = small_ps.tile([P, NDT, NB], F32, tag="small")
                    nc.tensor.matmul(
                        sbc_ps.rearrange("p a b -> p (a b)"),
                        ones_row,
                        srep.rearrange("p a b -> p (a b)"),
                        start=True, stop=True,
                    )
                    # h_k = s_k * pre_k, written into the (consumed) u slot
                    # k-1 of the slab: phase C batches the silu outputs later
                    hT = uT_slab[:, k - 1, :, :]
                    nc.vector.tensor_mul(hT, preT, sbc_ps)
                    nc.sync.dma_start(out=h_out[k], in_=hT)

                    hT_prev, pre_lp_prev, scol_prev = hT, pre_lp, scol

            # ---------------- Phase C: outs = h * silu(h), batched ----------------
            with (
                tc.tile_pool(name="phc", bufs=3) as phc,
            ):
                TCH = min(32, T_steps)  # timesteps per chunk
                for c0 in range(0, T_steps, TCH):
                    hch = uT_slab[:, c0:c0 + TCH, :, :]
                    sg = phc.tile([P, TCH, NDT, NB], F32, tag="sg")
                    nc.scalar.activation(
                        sg, hch, mybir.ActivationFunctionType.Sigmoid)
                    hsg = phc.tile([P, TCH, NDT, NB], F32, tag="hsg")
                    nc.vector.tensor_mul(hsg, hch, sg)
                    outc = phc.tile([P, TCH, NDT, NB], F32, tag="outc")
                    nc.vector.tensor_mul(outc, hsg, hch)
                    nc.sync.dma_start(
                        out=outs_out.rearrange("t p k b -> p t k b")[:, c0:c0 + TCH, :, :],
                        in_=outc)

    nc.compile()  # bacc legalization: ≤1 sync wait per instruction
    return nc


_NC_CACHE: dict = {}


def _get_nc(key, *args, **kwargs):
    if key not in _NC_CACHE:
        _NC_CACHE[key] = build_nc(*args, **kwargs)
    return _NC_CACHE[key]


def prepare_in_maps(x, h0, W, W_h, b, alpha, beta, lp_np=None, wh_scale: float = 1.0):
    """Host-side shard + transpose prep. Returns list of per-core input dicts."""
    import ml_dtypes
    if lp_np is None:
        lp_np = ml_dtypes.bfloat16
    T_steps = x.shape[0]
    WT_h = np.ascontiguousarray(W.T.astype(np.float32))
    WhT_h = np.ascontiguousarray((W_h.T * (beta * wh_scale)).astype(lp_np))
    abias_h = np.ascontiguousarray((alpha * b).astype(np.float32).reshape(D, 1))
    in_maps = []
    for c in range(NC):
        sl = slice(c * NB, (c + 1) * NB)
        xT_h = np.ascontiguousarray(x[:, sl, :].transpose(2, 0, 1).astype(np.float32))
        h0T_h = np.ascontiguousarray(h0[sl].T.astype(np.float32))
        in_maps.append({
            "xT": xT_h, "WT": WT_h, "WhT": WhT_h, "h0T": h0T_h, "abias": abias_h,
        })
    return in_maps


def assemble(results, T_steps):
    outs = np.empty((T_steps, B_FULL, D), np.float32)
    h = np.empty((T_steps + 1, B_FULL, D), np.float32)
    for c, r in enumerate(results):
        sl = slice(c * NB, (c + 1) * NB)
        outs[:, sl, :] = r["outs_out"].transpose(0, 3, 2, 1).reshape(T_steps, NB, D)
        h[:, sl, :] = r["h_out"].transpose(0, 3, 2, 1).reshape(T_steps + 1, NB, D)
    return outs, h


def run(x, h0, W, W_h, b, log_alpha, log_beta, trace=False, use_fp8=False):
    x = np.asarray(x, np.float32)
    h0 = np.asarray(h0, np.float32)
    W = np.asarray(W, np.float32)
    W_h = np.asarray(W_h, np.float32)
    b = np.asarray(b, np.float32)
    alpha = float(np.exp(np.float32(log_alpha)))
    beta = float(1.0 / (1.0 + math.exp(-float(log_beta))) * 0.1)
    T_steps = x.shape[0]

    import ml_dtypes
    if use_fp8:
        nc = _get_nc(("fp8", T_steps, alpha, beta), T_steps, alpha, beta,
                     lp_dtype=FP8, wh_scale=4096.0)
        in_maps = prepare_in_maps(x, h0, W, W_h, b, alpha, beta,
                                  lp_np=ml_dtypes.float8_e4m3, wh_scale=4096.0)
    else:
        nc = _get_nc(("v1", T_steps, alpha, beta), T_steps, alpha, beta)
        in_maps = prepare_in_maps(x, h0, W, W_h, b, alpha, beta)
    res = run_bass_kernel_spmd(nc, in_maps, list(range(NC)), trace=trace)
    outs, h = assemble(res.results, T_steps)
    return outs, h, res


def kernel(x, h0, W, W_h, b, log_alpha, log_beta):
    outs, h, _ = run(x, h0, W, W_h, b, log_alpha, log_beta)
    return outs, h


# revision 22
# speedup vs baseline: 1.5941x; 1.0387x over previous
"""Trainium2 Bass kernel for the MixedHighwayCell scan problem.

Reference computation (per timestep t, sequential over T=512):
    pre_t = h_{t-1} + alpha*wx_t + beta*(h_{t-1} @ W_h.T)
    h_t   = pre_t * rsqrt(mean(pre_t^2) + eps)
    out_t = h_t * silu(h_t)
with wx = x @ W.T + b precomputed for all t.

Strategy (8 cores, batch-sharded: 4 batch rows per core, no collectives):
  Phase A: uT[d, t, b] = alpha*(x @ W.T + b) computed in exact fp32,
           directly in transposed (d-major) layout, kept in SBUF.
  Phase B: sequential scan with the state kept UNNORMALIZED (pre_t) in
           d-major layout [128, 8, NB].  The rms scale s_t is pushed
           through the recurrent matmul:
               mm_t     = pre_{t-1} @ Wh_lp.T        (low precision OK:
                                                      result is scaled by beta)
               pre_t    = h_{t-1} + u_t + (beta*s_{t-1}) * mm_t
               h_{t-1}  = s_{t-1} * pre_{t-1}
           The matmul streams Wh^T (moving operand) against the thin
           stationary pre^T tiles; its b-major output is transposed back
           to d-major by PE-transposes whose "identity" operand is
           beta*s*I, fusing the scalar application for free.
"""

import math

import numpy as np

import concourse.bass as bass
import concourse.bacc as bacc
import concourse.tile as tile
from concourse import mybir
from concourse.bass_utils import run_bass_kernel_spmd

F32 = mybir.dt.float32
BF16 = mybir.dt.bfloat16
FP8 = mybir.dt.float8e4

P = 128          # partitions
D = 1024         # model dim
NDT = D // P     # d-tiles (8)
T_FULL = 512
B_FULL = 32
NC = 8           # cores
NB = B_FULL // NC  # batch rows per core (4)
EPS = 1e-6
ROWCHUNK = 512   # projection moving-chunk (rows of (t,b))


def build_nc(T_steps: int, alpha: float, beta: float, lp_dtype=BF16, wh_scale: float = 1.0):
    """Build the Bass program (identical SPMD program for all cores)."""
    nc = bacc.Bacc()

    xT = nc.declare_dram_parameter("xT", [D, T_steps, NB], F32, isOutput=False)
    WT = nc.declare_dram_parameter("WT", [D, D], F32, isOutput=False)
    WhT = nc.declare_dram_parameter("WhT", [D, D], lp_dtype, isOutput=False)
    h0T = nc.declare_dram_parameter("h0T", [D, NB], F32, isOutput=False)
    abias = nc.declare_dram_parameter("abias", [D, 1], F32, isOutput=False)
    h_out = nc.declare_dram_parameter("h_out", [T_steps + 1, P, NDT, NB], F32, isOutput=True)
    outs_out = nc.declare_dram_parameter("outs_out", [T_steps, P, NDT, NB], F32, isOutput=True)

    rows = T_steps * NB

    with tile.TileContext(nc) as tc:
        with (
            tc.tile_pool(name="slabs", bufs=1) as slabs,
            tc.tile_pool(name="projw", bufs=1) as projw,
            tc.tile_pool(name="projx", bufs=2) as projx,
            tc.tile_pool(name="projps", bufs=2, space="PSUM") as projps,
        ):
            # persistent SBUF slabs
            wh_slab = slabs.tile([P, NDT, D], lp_dtype)
            WhT_view = WhT.rearrange("(k p) e -> p k e", p=P)
            for kt in range(NDT):
                nc.sync.dma_start(out=wh_slab[:, kt, :], in_=WhT_view[:, kt, :])
            uT_slab = slabs.tile([P, T_steps, NDT, NB], F32)
            abias_sb = slabs.tile([P, NDT], F32)
            nc.sync.dma_start(out=abias_sb, in_=abias.rearrange("(k p) one -> p (k one)", p=P))

            # scan constants (init before the DMA-heavy projection so these
            # memsets don't inherit WAR waits against all 8 DMA queues)
            ones_col = slabs.tile([P, 1], F32)
            nc.vector.memset(ones_col, 1.0)
            ones_row = slabs.tile([1, P], F32)
            nc.vector.memset(ones_row, 1.0)
            one_1x1 = slabs.tile([1, 1], F32)
            nc.vector.memset(one_1x1, 1.0)
            eps_ap = slabs.tile([1, 1], F32)
            nc.vector.memset(eps_ap, EPS)
            prime_d = slabs.tile([P, 1], F32)
            nc.vector.tensor_copy(prime_d, abias_sb[:, 0:1])  # DVE observes abias DMA
            ident_lp = slabs.tile([NB, NB], BF16)
            nc.vector.memset(ident_lp, 0.0)
            nc.gpsimd.affine_select(
                out=ident_lp, in_=ident_lp,
                compare_op=mybir.AluOpType.not_equal,
                fill=1.0,
                base=0, pattern=[[-1, NB]], channel_multiplier=1,
            )

            # ---------------- Phase A: projection  uT = alpha*(x @ W.T + b) ----------------
            # projection SBUF pools stay open for the whole kernel: reusing
            # their DMA-written SBUF would attach WAR waits against all 8 DMA
            # queues to later instructions (walrus wait-count limit).
            if True:
                wt_slab = projw.tile([P, NDT, D], F32)
                WT_view = WT.rearrange("(k p) e -> p k e", p=P)
                for kt in range(NDT):
                    nc.sync.dma_start(out=wt_slab[:, kt, :], in_=WT_view[:, kt, :])

                xT_view = xT.rearrange("(k p) t b -> p k (t b)", p=P)
                off = 0
                while off < rows:
                    rc = min(ROWCHUNK, rows - off)
                    t0, nt = off // NB, rc // NB
                    xt = projx.tile([P, NDT, rc], F32, tag="xt")
                    for kt in range(NDT):
                        nc.sync.dma_start(out=xt[:, kt, :], in_=xT_view[:, kt, off:off + rc])
                    for et in range(NDT):
                        ps = projps.tile([P, rc], F32, tag="ps")
                        if off >= 2 * ROWCHUNK or et >= 2:
                            # dummy PE write into the slot: funnels the WAR
                            # edge (vs the DVE uT-copy that read this slot)
                            # into one PE instruction with a single wait
                            prev_t0 = max(0, t0 - (2 * ROWCHUNK) // NB) if et < 2 else t0
                            nc.tensor.transpose(
                                ps[0:NDT, 0:1],
                                uT_slab[0:1, prev_t0, :, 0],
                                one_1x1,
                            )
                        for kt in range(NDT):
                            nc.tensor.matmul(
                                ps,
                                wt_slab[:, kt, et * P:(et + 1) * P],
                                xt[:, kt, :],
                                start=(kt == 0),
                                stop=(kt == NDT - 1),
                            )
                        # uT[:, t0:t0+nt, et, :] = alpha*ps + abias[et]  (DVE:
                        # the ACT struct allows only one semaphore wait)
                        nc.vector.tensor_scalar(
                            out=uT_slab[:, t0:t0 + nt, et, :],
                            in0=ps.rearrange("p (t b) -> p t b", b=NB),
                            scalar1=float(alpha),
                            scalar2=abias_sb[:, et:et + 1],
                            op0=mybir.AluOpType.mult,
                            op1=mybir.AluOpType.add,
                        )
                    off += rc

            # ---------------- Phase B: the scan ----------------
            with (
                tc.tile_pool(name="state", bufs=2) as state,
                tc.tile_pool(name="work", bufs=2) as work,
                tc.tile_pool(name="mmps", bufs=1, space="PSUM") as mmps_pool,
                tc.tile_pool(name="mmtps", bufs=2, space="PSUM") as mmt_pool,
                tc.tile_pool(name="smallps", bufs=2, space="PSUM") as small_ps,
            ):
                use_fp8 = lp_dtype == FP8
                lp_cols = 16 if use_fp8 else NB
                # initial state: pre_0 = h0, s_0 = 1
                hT_prev = state.tile([P, NDT, NB], F32, tag="hT")
                nc.sync.dma_start(out=hT_prev, in_=h0T.rearrange("(k p) b -> p k b", p=P))
                pre_lp_prev = state.tile([P, NDT, lp_cols], lp_dtype, tag="prelp")
                nc.scalar.copy(pre_lp_prev[:, :, 0:NB], hT_prev)
                scol_prev = state.tile([NB, 1], F32, tag="scol")
                nc.scalar.activation(
                    scol_prev, ones_col[0:NB, :],
                    mybir.ActivationFunctionType.Copy, scale=1.0 / wh_scale,
                )  # s_0 = 1
                nc.sync.dma_start(out=h_out[0], in_=hT_prev)

                NH = D // 512  # moving chunks of the recurrent matmul (2)
                for k in range(1, T_steps + 1):
                    # (1) mm = pre_{k-1} @ Wh^T   [NB, D] in PSUM (b-major)
                    mm_ps = mmps_pool.tile([NB, D], F32, tag="mm")
                    if use_fp8:
                        # DoubleRow: contraction 256 per matmul over jt-pairs
                        for nch in range(NH):
                            for kt in range(NDT // 2):
                                nc.tensor.matmul(
                                    mm_ps[:, nch * 512:(nch + 1) * 512],
                                    pre_lp_prev[:, 2 * kt:2 * kt + 2, 0:NB],
                                    wh_slab[:, 2 * kt:2 * kt + 2, nch * 512:(nch + 1) * 512],
                                    start=(kt == 0),
                                    stop=(kt == NDT // 2 - 1),
                                    perf_mode=mybir.MatmulPerfMode.DoubleRow,
                                )
                    else:
                        for nch in range(NH):
                            for kt in range(NDT):
                                nc.tensor.matmul(
                                    mm_ps[:, nch * 512:(nch + 1) * 512],
                                    pre_lp_prev[:, kt, :],
                                    wh_slab[:, kt, nch * 512:(nch + 1) * 512],
                                    start=(kt == 0),
                                    stop=(kt == NDT - 1),
                                )
                    # (2) scale by s_{k-1}/wh_scale and cast to bf16 (DVE:
                    # keeps the ACT engine free of table switches)
                    mm_sb = work.tile([NB, NDT, P], BF16, tag="mmsb")
                    for nch in range(NH):
                        nc.vector.tensor_scalar_mul(
                            mm_sb[:, nch * 4:(nch + 1) * 4, :],
                            mm_ps[:, nch * 512:(nch + 1) * 512].rearrange(
                                "p (a c) -> p a c", c=P),
                            scol_prev,
                        )
                    # (3) transpose back to d-major, scaled by beta*s_{k-1}
                    mmT_ps = mmt_pool.tile([P, NDT, NB], BF16, tag="mmt")
                    for jt in range(NDT):
                        nc.tensor.transpose(mmT_ps[:, jt, :], mm_sb[:, jt, :], ident_lp)
                    # (4) pre_k = h_{k-1} + u_k + mmT
                    tmp = work.tile([P, NDT, NB], F32, tag="tmp")
                    nc.vector.tensor_add(tmp, hT_prev, uT_slab[:, k - 1, :, :])
                    preT = work.tile([P, NDT, NB], F32, tag="pre")
                    nc.vector.tensor_add(preT, tmp, mmT_ps)
                    # (5) cast for the next matmul (critical path!)
                    pre_lp = state.tile([P, NDT, lp_cols], lp_dtype, tag="prelp")
                    nc.vector.tensor_copy(pre_lp[:, :, 0:NB], preT)

                    # ---- norm branch (off critical path) ----
                    sq = work.tile([P, NDT, NB], F32, tag="sq")
                    nc.vector.tensor_mul(sq, preT, preT)
                    ssq_ps = small_ps.tile([1, NDT, NB], F32, tag="small")
                    nc.tensor.matmul(
                        ssq_ps.rearrange("p a b -> p (a b)"),
                        ones_col,
                        sq.rearrange("p a b -> p (a b)"),
                        start=True, stop=True,
                    )
                    ssq_r = work.tile([1, NB], F32, tag="ssqr")
                    nc.vector.tensor_reduce(
                        ssq_r, ssq_ps.rearrange("p a b -> p b a"),
                        axis=mybir.AxisListType.X, op=mybir.AluOpType.add,
                    )
                    s_tmp = work.tile([1, NB], F32, tag="stmp")
                    nc.scalar.activation(
                        s_tmp, ssq_r, mybir.ActivationFunctionType.Sqrt,
                        bias=eps_ap, scale=1.0 / D,
                    )
                    s_row = work.tile([1, NB], F32, tag="srow")
                    nc.vector.reciprocal(s_row, s_tmp)
                    # replicate s_row 8x along free for the broadcast matmul
                    srep = work.tile([1, NDT, NB], F32, tag="srep")
                    nc.vector.tensor_copy(
                        out=srep,
                        in_=bass.AP(
                            tensor=s_row.tensor, offset=s_row.offset,
                            ap=[s_row.ap[0], [0, NDT], s_row.ap[-1]],
                        ),
                    )
                    # s as a column (for the diag build)
                    scol_ps = small_ps.tile([NB, 1], F32, tag="small")
                    nc.tensor.transpose(scol_ps, s_row, one_1x1)
                    scol = state.tile([NB, 1], F32, tag="scol")
                    nc.vector.tensor_scalar_mul(scol, scol_ps, 1.0 / wh_scale)
                    # broadcast s across partitions: sbc = ones_col128 x srep
                    sbc_ps = small_ps.tile([P, NDT, NB], F32, tag="small")
                    nc.tensor.matmul(
                        sbc_ps.rearrange("p a b -> p (a b)"),
                        ones_row,
                        srep.rearrange("p a b -> p (a b)"),
                        start=True, stop=True,
                    )
                    # h_k = s_k * pre_k, written into the (consumed) u slot
                    # k-1 of the slab: phase C batches the silu outputs later
                    hT = uT_slab[:, k - 1, :, :]
                    nc.vector.tensor_mul(hT, preT, sbc_ps)
                    nc.sync.dma_start(out=h_out[k], in_=hT)

                    hT_prev, pre_lp_prev, scol_prev = hT, pre_lp, scol

            # ---------------- Phase C: outs = h * silu(h), batched ----------------
            with (
                tc.tile_pool(name="phc", bufs=3) as phc,
            ):
                TCH = min(32, T_steps)  # timesteps per chunk
                for c0 in range(0, T_steps, TCH):
                    hch = uT_slab[:, c0:c0 + TCH, :, :]
                    sg = phc.tile([P, TCH, NDT, NB], F32, tag="sg")
                    nc.scalar.activation(
                        sg, hch, mybir.ActivationFunctionType.Sigmoid)
                    hsg = phc.tile([P, TCH, NDT, NB], F32, tag="hsg")
                    nc.vector.tensor_mul(hsg, hch, sg)
                    outc = phc.tile([P, TCH, NDT, NB], F32, tag="outc")
                    nc.vector.tensor_mul(outc, hsg, hch)
                    nc.sync.dma_start(
                        out=outs_out.rearrange("t p k b -> p t k b")[:, c0:c0 + TCH, :, :],
                        in_=outc)

    nc.compile()  # bacc legalization: ≤1 sync wait per instruction
    return nc


def build_nc2(T_steps: int, alpha: float, beta: float):
    """2-step scan: stream G=(I+bWh)^2-I every other step; odd steps are
    reconstructed in a batched phase afterwards from the scalar chain."""
    assert T_steps % 2 == 0
    nc = bacc.Bacc()
    TH = T_steps // 2

    xT = nc.declare_dram_parameter("xT", [D, T_steps, NB], F32, isOutput=False)
    WT = nc.declare_dram_parameter("WT", [D, D], F32, isOutput=False)
    WhT = nc.declare_dram_parameter("WhT", [D, D], BF16, isOutput=False)  # beta*Wh^T
    GT = nc.declare_dram_parameter("GT", [D, D], BF16, isOutput=False)    # G^T
    h0T = nc.declare_dram_parameter("h0T", [D, NB], F32, isOutput=False)
    abias = nc.declare_dram_parameter("abias", [D, 1], F32, isOutput=False)
    h_out = nc.declare_dram_parameter("h_out", [T_steps + 1, P, NDT, NB], F32, isOutput=True)
    outs_out = nc.declare_dram_parameter("outs_out", [T_steps, P, NDT, NB], F32, isOutput=True)

    rows = T_steps * NB

    with tile.TileContext(nc) as tc:
        with (
            tc.tile_pool(name="slabs", bufs=1) as slabs,
        ):
            wh_slab = slabs.tile([P, NDT, D], BF16)
            WhT_view = WhT.rearrange("(k p) e -> p k e", p=P)
            for kt in range(NDT):
                nc.sync.dma_start(out=wh_slab[:, kt, :], in_=WhT_view[:, kt, :])
            gt_slab = slabs.tile([P, NDT, D], BF16)
            GT_view = GT.rearrange("(k p) e -> p k e", p=P)
            for kt in range(NDT):
                nc.sync.dma_start(out=gt_slab[:, kt, :], in_=GT_view[:, kt, :])
            uT_slab = slabs.tile([P, T_steps, NDT, NB], F32)
            vT_slab = slabs.tile([P, TH, NDT, NB], F32)
            un2_slab = slabs.tile([1, T_steps, NB], F32)
            s1_slab = slabs.tile([1, TH, NB], F32)
            abias_sb = slabs.tile([P, NDT], F32)
            nc.sync.dma_start(out=abias_sb, in_=abias.rearrange("(k p) one -> p (k one)", p=P))

            ones_col = slabs.tile([P, 1], F32)
            nc.vector.memset(ones_col, 1.0)
            ones_row = slabs.tile([1, P], F32)
            nc.vector.memset(ones_row, 1.0)
            one_1x1 = slabs.tile([1, 1], F32)
            nc.vector.memset(one_1x1, 1.0)
            eps_ap = slabs.tile([1, 1], F32)
            nc.vector.memset(eps_ap, EPS)
            ident_lp = slabs.tile([NB, NB], BF16)
            nc.vector.memset(ident_lp, 0.0)
            nc.gpsimd.affine_select(
                out=ident_lp, in_=ident_lp,
                compare_op=mybir.AluOpType.not_equal,
                fill=1.0, base=0, pattern=[[-1, NB]], channel_multiplier=1,
            )

            # ---------------- Phase A: uT = alpha*(x @ W.T + b) ----------------
            with (
                tc.tile_pool(name="projw", bufs=1) as projw,
                tc.tile_pool(name="projx", bufs=2) as projx,
                tc.tile_pool(name="projps", bufs=2, space="PSUM") as projps,
            ):
                wt_slab = projw.tile([P, NDT, D], F32)
                WT_view = WT.rearrange("(k p) e -> p k e", p=P)
                for kt in range(NDT):
                    nc.sync.dma_start(out=wt_slab[:, kt, :], in_=WT_view[:, kt, :])
                xT_view = xT.rearrange("(k p) t b -> p k (t b)", p=P)
                off = 0
                while off < rows:
                    rc = min(ROWCHUNK, rows - off)
                    t0, nt = off // NB, rc // NB
                    xt = projx.tile([P, NDT, rc], F32, tag="xt")
                    for kt in range(NDT):
                        nc.sync.dma_start(out=xt[:, kt, :], in_=xT_view[:, kt, off:off + rc])
                    for et in range(NDT):
                        ps = projps.tile([P, rc], F32, tag="ps")
                        for kt in range(NDT):
                            nc.tensor.matmul(
                                ps, wt_slab[:, kt, et * P:(et + 1) * P], xt[:, kt, :],
                                start=(kt == 0), stop=(kt == NDT - 1))
                        nc.vector.tensor_scalar(
                            out=uT_slab[:, t0:t0 + nt, et, :],
                            in0=ps.rearrange("p (t b) -> p t b", b=NB),
                            scalar1=float(alpha), scalar2=abias_sb[:, et:et + 1],
                            op0=mybir.AluOpType.mult, op1=mybir.AluOpType.add)
                    off += rc

            # ---------------- Phase A2: v, ||u||^2, lp cast of u ----------------
            with (
                tc.tile_pool(name="a2sb", bufs=1) as a2sb,
                tc.tile_pool(name="a2ps", bufs=2, space="PSUM") as a2ps,
            ):
                # ||u_t||^2 for all t
                tt0 = 0
                while tt0 < T_steps:
                    tch = min(16, T_steps - tt0)
                    usq = a2sb.tile([P, tch, NDT, NB], F32, tag="usq")
                    nc.vector.tensor_mul(
                        usq, uT_slab[:, tt0:tt0 + tch, :, :], uT_slab[:, tt0:tt0 + tch, :, :])
                    ups = a2ps.tile([1, tch, NDT, NB], F32, tag="ups")
                    els = tch * NDT * NB
                    for q0 in range(0, els, 512):
                        q1 = min(q0 + 512, els)
                        nc.tensor.matmul(
                            ups.rearrange("p t k b -> p (t k b)")[:, q0:q1],
                            ones_col,
                            usq.rearrange("p t k b -> p (t k b)")[:, q0:q1],
                            start=True, stop=True)
                    nc.vector.tensor_reduce(
                        un2_slab[:, tt0:tt0 + tch, :],
                        ups.rearrange("p t k b -> p t b k"),
                        axis=mybir.AxisListType.X, op=mybir.AluOpType.add)
                    tt0 += tch
                # bf16 copy of u at even steps, kt-major for the v matmul
                ub = a2sb.tile([P, NDT, TH * NB], BF16, tag="ub")
                for kt in range(NDT):
                    nc.vector.tensor_copy(
                        ub[:, kt, :].rearrange("p (t b) -> p t b", b=NB),
                        uT_slab[:, 0:T_steps:2, kt, :])
                # v = u + u @ (beta*Wh)^T  for even slots
                VR = TH * NB  # rows
                nvch = (VR + 511) // 512
                for et in range(NDT):
                    for vc in range(nvch):
                        r0 = vc * 512
                        rch = min(512, VR - r0)
                        vps = a2ps.tile([P, 512], F32, tag="vps")
                        for kt in range(NDT):
                            nc.tensor.matmul(
                                vps[:, 0:rch],
                                wh_slab[:, kt, et * P:(et + 1) * P],
                                ub[:, kt, r0:r0 + rch],
                                start=(kt == 0), stop=(kt == NDT - 1))
                        t0v, t1v = r0 // NB, (r0 + rch) // NB
                        nc.vector.tensor_add(
                            vT_slab[:, t0v:t1v, et, :],
                            vps[:, 0:rch].rearrange("p (t b) -> p t b", b=NB),
                            uT_slab[:, 2 * t0v:2 * t1v:2, et, :])

            # ---------------- Phase B: 2-step scan ----------------
            with (
                tc.tile_pool(name="state", bufs=2) as state,
                tc.tile_pool(name="work", bufs=2) as work,
                tc.tile_pool(name="mmps", bufs=1, space="PSUM") as mmps_pool,
                tc.tile_pool(name="mmtps", bufs=2, space="PSUM") as mmt_pool,
                tc.tile_pool(name="smallps", bufs=4, space="PSUM") as small_ps,
            ):
                # init: pre_0 = h0, s_0 = 1, sigma_0^2 = ||h0||^2
                hT0 = slabs.tile([P, NDT, NB], F32, tag="hT0")
                nc.sync.dma_start(out=hT0, in_=h0T.rearrange("(k p) b -> p k b", p=P))
                pre_prev = hT0  # pre_0 tile (f32)
                h_prev = hT0    # h_0 = pre_0
                pre_lp_prev = state.tile([P, NDT, NB], BF16, tag="prelp")
                nc.vector.tensor_copy(pre_lp_prev, hT0)
                scol_prev = state.tile([NB, 1], F32, tag="scol")
                nc.vector.memset(scol_prev, 1.0)
                srow_prev = state.tile([1, NB], F32, tag="srow")
                nc.vector.memset(srow_prev, 1.0)
                sq0 = work.tile([P, NDT, NB], F32, tag="sq")
                nc.vector.tensor_mul(sq0, hT0, hT0)
                ssq0_ps = small_ps.tile([1, NDT, NB], F32, tag="small")
                nc.tensor.matmul(
                    ssq0_ps.rearrange("p a b -> p (a b)"), ones_col,
                    sq0.rearrange("p a b -> p (a b)"), start=True, stop=True)
                sig_prev = work.tile([1, NB], F32, tag="sig")
                nc.vector.tensor_reduce(
                    sig_prev, ssq0_ps.rearrange("p a b -> p b a"),
                    axis=mybir.AxisListType.X, op=mybir.AluOpType.add)
                nc.sync.dma_start(out=h_out[0], in_=hT0)

                for it in range(TH):
                    s = 2 * it  # current even step index (pre_s held)
                    # (1) g = pre_s @ G^T
                    mm_ps = mmps_pool.tile([NB, D], F32, tag="mm")
                    for nch in range(2):
                        for kt in range(NDT):
                            nc.tensor.matmul(
                                mm_ps[:, nch * 512:(nch + 1) * 512],
                                pre_lp_prev[:, kt, :],
                                gt_slab[:, kt, nch * 512:(nch + 1) * 512],
                                start=(kt == 0), stop=(kt == NDT - 1))
                    # (2) scale by s_s, cast bf16
                    mm_sb = work.tile([NB, NDT, P], BF16, tag="mmsb")
                    for nch in range(2):
                        nc.vector.tensor_scalar_mul(
                            mm_sb[:, nch * 4:(nch + 1) * 4, :],
                            mm_ps[:, nch * 512:(nch + 1) * 512].rearrange(
                                "p (a c) -> p a c", c=P),
                            scol_prev)
                    # (3) transpose to d-major: sgT = s_s * g  (bf16)
                    sgT_ps = mmt_pool.tile([P, NDT, NB], BF16, tag="mmt")
                    for jt in range(NDT):
                        nc.tensor.transpose(sgT_ps[:, jt, :], mm_sb[:, jt, :], ident_lp)
                    # (4) scalars: dt1 = <s*g, pre>, dt2 = <pre, u_{s+1}>
                    q1 = work.tile([P, NDT, NB], F32, tag="q1")
                    nc.vector.tensor_mul(q1, sgT_ps, pre_prev)
                    q2 = work.tile([P, NDT, NB], F32, tag="q2")
                    nc.vector.tensor_mul(q2, pre_prev, uT_slab[:, s, :, :])
                    d1_ps = small_ps.tile([1, NDT, NB], F32, tag="small")
                    nc.tensor.matmul(
                        d1_ps.rearrange("p a b -> p (a b)"), ones_col,
                        q1.rearrange("p a b -> p (a b)"), start=True, stop=True)
                    d2_ps = small_ps.tile([1, NDT, NB], F32, tag="small")
                    nc.tensor.matmul(
                        d2_ps.rearrange("p a b -> p (a b)"), ones_col,
                        q2.rearrange("p a b -> p (a b)"), start=True, stop=True)
                    dt1 = work.tile([1, NB], F32, tag="dt1")
                    nc.vector.tensor_reduce(
                        dt1, d1_ps.rearrange("p a b -> p b a"),
                        axis=mybir.AxisListType.X, op=mybir.AluOpType.add)
                    dt2 = work.tile([1, NB], F32, tag="dt2")
                    nc.vector.tensor_reduce(
                        dt2, d2_ps.rearrange("p a b -> p b a"),
                        axis=mybir.AxisListType.X, op=mybir.AluOpType.add)
                    # (5) sigma_{s+1}^2 = s^2*sig + s*dt1 + 2*s*dt2 + un2[s]
                    t1 = work.tile([1, NB], F32, tag="t1")
                    nc.vector.scalar_tensor_tensor(
                        t1, in0=dt2, scalar=2.0, in1=dt1,
                        op0=mybir.AluOpType.mult, op1=mybir.AluOpType.add)
                    t2 = work.tile([1, NB], F32, tag="t2")
                    nc.vector.tensor_mul(t2, srow_prev, sig_prev)
                    t3 = work.tile([1, NB], F32, tag="t3")
                    nc.vector.tensor_add(t3, t2, t1)   # s*sig^2... see note
                    t4 = work.tile([1, NB], F32, tag="t4")
                    nc.vector.tensor_mul(t4, srow_prev, t3)
                    sig1 = work.tile([1, NB], F32, tag="sig1")
                    nc.vector.tensor_add(sig1, t4, un2_slab[:, s, :])
                    # NOTE: s^2*sig + s*(dt1+2dt2) = s*(s*sig + dt1 + 2*dt2)
                    s1t = work.tile([1, NB], F32, tag="s1t")
                    nc.scalar.activation(
                        s1t, sig1, mybir.ActivationFunctionType.Sqrt,
                        bias=eps_ap, scale=1.0 / D)
                    s1_row = work.tile([1, NB], F32, tag="s1row")
                    nc.vector.reciprocal(s1_row, s1t)
                    nc.vector.tensor_copy(s1_slab[:, it, :], s1_row)
                    # (6) broadcast s_{s+1} across partitions
                    s1rep = work.tile([1, NDT, NB], F32, tag="s1rep")
                    nc.vector.tensor_copy(
                        out=s1rep,
                        in_=bass.AP(tensor=s1_row.tensor, offset=s1_row.offset,
                                    ap=[s1_row.ap[0], [0, NDT], s1_row.ap[-1]]))
                    sbc1_ps = small_ps.tile([P, NDT, NB], F32, tag="small")
                    nc.tensor.matmul(
                        sbc1_ps.rearrange("p a b -> p (a b)"), ones_row,
                        s1rep.rearrange("p a b -> p (a b)"), start=True, stop=True)
                    # (7) inner = h_s + s*g + v_{s+1};  pre_{s+2} = s_{s+1}*inner + u_{s+2}
                    inner1 = work.tile([P, NDT, NB], F32, tag="inner1")
                    nc.vector.tensor_add(inner1, h_prev, vT_slab[:, it, :, :])
                    inner = work.tile([P, NDT, NB], F32, tag="inner")
                    nc.vector.tensor_add(inner, inner1, sgT_ps)
                    pre2a = work.tile([P, NDT, NB], F32, tag="pre2a")
                    nc.vector.tensor_mul(pre2a, inner, sbc1_ps)
                    preT = work.tile([P, NDT, NB], F32, tag="pre")
                    nc.vector.tensor_add(preT, pre2a, uT_slab[:, s + 1, :, :])
                    pre_lp = state.tile([P, NDT, NB], BF16, tag="prelp")
                    nc.vector.tensor_copy(pre_lp, preT)
                    # (8) sigma_{s+2}, s_{s+2}
                    sq = work.tile([P, NDT, NB], F32, tag="sq")
                    nc.vector.tensor_mul(sq, preT, preT)
                    ssq_ps = small_ps.tile([1, NDT, NB], F32, tag="small")
                    nc.tensor.matmul(
                        ssq_ps.rearrange("p a b -> p (a b)"), ones_col,
                        sq.rearrange("p a b -> p (a b)"), start=True, stop=True)
                    sig2 = work.tile([1, NB], F32, tag="sig")
                    nc.vector.tensor_reduce(
                        sig2, ssq_ps.rearrange("p a b -> p b a"),
                        axis=mybir.AxisListType.X, op=mybir.AluOpType.add)
                    s2t = work.tile([1, NB], F32, tag="s2t")
                    nc.scalar.activation(
                        s2t, sig2, mybir.ActivationFunctionType.Sqrt,
                        bias=eps_ap, scale=1.0 / D)
                    s2_row = work.tile([1, NB], F32, tag="srow")
                    nc.vector.reciprocal(s2_row, s2t)
                    s2rep = work.tile([1, NDT, NB], F32, tag="s2rep")
                    nc.vector.tensor_copy(
                        out=s2rep,
                        in_=bass.AP(tensor=s2_row.tensor, offset=s2_row.offset,
                                    ap=[s2_row.ap[0], [0, NDT], s2_row.ap[-1]]))
                    sbc2_ps = small_ps.tile([P, NDT, NB], F32, tag="small")
                    nc.tensor.matmul(
                        sbc2_ps.rearrange("p a b -> p (a b)"), ones_row,
                        s2rep.rearrange("p a b -> p (a b)"), start=True, stop=True)
                    scol_ps = small_ps.tile([NB, 1], F32, tag="small")
                    nc.tensor.transpose(scol_ps, s2_row, one_1x1)
                    scol = state.tile([NB, 1], F32, tag="scol")
                    nc.vector.tensor_scalar_mul(scol, scol_ps, 1.0)
                    # (9) h_{s+2} into slab slot s+1
                    hT = uT_slab[:, s + 1, :, :]
                    nc.vector.tensor_mul(hT, preT, sbc2_ps)
                    nc.sync.dma_start(out=h_out[s + 2], in_=hT)

                    srow_new = state.tile([1, NB], F32, tag="srow")
                    nc.vector.tensor_copy(srow_new, s2_row)
                    pre_prev, h_prev, pre_lp_prev = preT, hT, pre_lp
                    scol_prev, srow_prev, sig_prev = scol, srow_new, sig2

            # ---------------- Phase D: reconstruct odd steps ----------------
            with (
                tc.tile_pool(name="phd", bufs=1) as phd,
                tc.tile_pool(name="phdw", bufs=3) as phdw,
                tc.tile_pool(name="phdps", bufs=4, space="PSUM") as phdps,
            ):
                # bf16 copy of even h's, kt-major: rows = (s_even, b)
                hb = phd.tile([P, NDT, TH * NB], BF16)
                for kt in range(NDT):
                    nc.vector.tensor_copy(hb[:, kt, 0:NB], hT0[:, kt, :])
                    nc.vector.tensor_copy(
                        hb[:, kt, NB:].rearrange("p (t b) -> p t b", b=NB),
                        uT_slab[:, 1:T_steps - 1:2, kt, :])
                VR = TH * NB
                nvch = (VR + 511) // 512
                for et in range(NDT):
                    for vc in range(nvch):
                        r0 = vc * 512
                        rch = min(512, VR - r0)
                        dps = phdps.tile([P, 512], F32, tag="dps")
                        for kt in range(NDT):
                            nc.tensor.matmul(
                                dps[:, 0:rch],
                                wh_slab[:, kt, et * P:(et + 1) * P],
                                hb[:, kt, r0:r0 + rch],
                                start=(kt == 0), stop=(kt == NDT - 1))
                        # pre_odd = h_even + mm + u_odd ; h_odd = s1 * pre_odd
                        tmp = phdw.tile([P, 512], F32, tag="tmp")
                        # h_even rows r0..r0+rch in f32: slot s-1 for s>=2, hT0 for s=0
                        t_lo = r0 // NB
                        t_hi = (r0 + rch) // NB
                        # assemble in two parts if the h0 row-block is included
                        parts = []
                        if t_lo == 0:
                            parts.append((0, 1, hT0[:, et:et + 1, :]))
                            parts.append((1, t_hi,
                                          uT_slab[:, 1:2 * t_hi - 2:2, et, :]))
                        else:
                            parts.append((t_lo, t_hi,
                                          uT_slab[:, 2 * t_lo - 1:2 * t_hi - 2:2, et, :]))
                        for (ta, tb_, hsrc) in parts:
                            o0 = (ta - t_lo) * NB
                            o1 = (tb_ - t_lo) * NB
                            nc.vector.tensor_add(
                                tmp[:, o0:o1].rearrange("p (t b) -> p t b", b=NB),
                                hsrc,
                                uT_slab[:, 2 * ta:2 * tb_:2, et, :])
                        pre_o = phdw.tile([P, 512], F32, tag="preo")
                        nc.vector.tensor_add(pre_o[:, 0:rch], tmp[:, 0:rch], dps[:, 0:rch])
                        # broadcast s1 over partitions for these rows
                        sbc_ps = phdps.tile([P, 512], F32, tag="sbc")
                        nc.tensor.matmul(
                            sbc_ps[:, 0:rch], ones_row,
                            s1_slab.rearrange("p t b -> p (t b)")[:, r0:r0 + rch],
                            start=True, stop=True)
                        ho = phdw.tile([P, 512], F32, tag="ho")
                        nc.vector.tensor_mul(ho[:, 0:rch], pre_o[:, 0:rch], sbc_ps[:, 0:rch])
                        # store into slab slots s (odd h at slot s) and DMA
                        nc.vector.tensor_copy(
                            uT_slab[:, 2 * t_lo:2 * t_hi:2, et, :],
                            ho[:, 0:rch].rearrange("p (t b) -> p t b", b=NB))
                for s_even in range(0, T_steps, 2):
                    nc.sync.dma_start(
                        out=h_out[s_even + 1], in_=uT_slab[:, s_even, :, :])

            # ---------------- Phase C: outs = h * silu(h) ----------------
            with (
                tc.tile_pool(name="phc", bufs=3) as phc,
            ):
                TCH = min(32, T_steps)
                for c0 in range(0, T_steps, TCH):
                    hch = uT_slab[:, c0:c0 + TCH, :, :]
                    sg = phc.tile([P, TCH, NDT, NB], F32, tag="sg")
                    nc.scalar.activation(sg, hch, mybir.ActivationFunctionType.Sigmoid)
                    hsg = phc.tile([P, TCH, NDT, NB], F32, tag="hsg")
                    nc.vector.tensor_mul(hsg, hch, sg)
                    outc = phc.tile([P, TCH, NDT, NB], F32, tag="outc")
                    nc.vector.tensor_mul(outc, hsg, hch)
                    nc.sync.dma_start(
                        out=outs_out.rearrange("t p k b -> p t k b")[:, c0:c0 + TCH, :, :],
                        in_=outc)

    nc.compile()
    return nc


_NC_CACHE: dict = {}


def _get_nc(key, *args, **kwargs):
    if key not in _NC_CACHE:
        builder = build_nc
        if args and callable(args[0]):
            builder, args = args[0], args[1:]
        _NC_CACHE[key] = builder(*args, **kwargs)
    return _NC_CACHE[key]


def prepare_in_maps(x, h0, W, W_h, b, alpha, beta, lp_np=None, wh_scale: float = 1.0):
    """Host-side shard + transpose prep. Returns list of per-core input dicts."""
    import ml_dtypes
    if lp_np is None:
        lp_np = ml_dtypes.bfloat16
    T_steps = x.shape[0]
    WT_h = np.ascontiguousarray(W.T.astype(np.float32))
    WhT_h = np.ascontiguousarray((W_h.T * (beta * wh_scale)).astype(lp_np))
    abias_h = np.ascontiguousarray((alpha * b).astype(np.float32).reshape(D, 1))
    in_maps = []
    for c in range(NC):
        sl = slice(c * NB, (c + 1) * NB)
        xT_h = np.ascontiguousarray(x[:, sl, :].transpose(2, 0, 1).astype(np.float32))
        h0T_h = np.ascontiguousarray(h0[sl].T.astype(np.float32))
        in_maps.append({
            "xT": xT_h, "WT": WT_h, "WhT": WhT_h, "h0T": h0T_h, "abias": abias_h,
        })
    return in_maps


def assemble(results, T_steps):
    outs = np.empty((T_steps, B_FULL, D), np.float32)
    h = np.empty((T_steps + 1, B_FULL, D), np.float32)
    for c, r in enumerate(results):
        sl = slice(c * NB, (c + 1) * NB)
        outs[:, sl, :] = r["outs_out"].transpose(0, 3, 2, 1).reshape(T_steps, NB, D)
        h[:, sl, :] = r["h_out"].transpose(0, 3, 2, 1).reshape(T_steps + 1, NB, D)
    return outs, h


def prepare_in_maps2(x, h0, W, W_h, b, alpha, beta):
    import ml_dtypes
    WT_h = np.ascontiguousarray(W.T.astype(np.float32))
    WhT_h = np.ascontiguousarray((beta * W_h.T).astype(ml_dtypes.bfloat16))
    G = (2.0 * beta) * W_h + (beta * beta) * (W_h.astype(np.float64) @ W_h.astype(np.float64)).astype(np.float32)
    GT_h = np.ascontiguousarray(G.T.astype(ml_dtypes.bfloat16))
    abias_h = np.ascontiguousarray((alpha * b).astype(np.float32).reshape(D, 1))
    in_maps = []
    for c in range(NC):
        sl = slice(c * NB, (c + 1) * NB)
        xT_h = np.ascontiguousarray(x[:, sl, :].transpose(2, 0, 1).astype(np.float32))
        h0T_h = np.ascontiguousarray(h0[sl].T.astype(np.float32))
        in_maps.append({
            "xT": xT_h, "WT": WT_h, "WhT": WhT_h, "GT": GT_h,
            "h0T": h0T_h, "abias": abias_h,
        })
    return in_maps


def run(x, h0, W, W_h, b, log_alpha, log_beta, trace=False, use_fp8=False):
    x = np.asarray(x, np.float32)
    h0 = np.asarray(h0, np.float32)
    W = np.asarray(W, np.float32)
    W_h = np.asarray(W_h, np.float32)
    b = np.asarray(b, np.float32)
    alpha = float(np.exp(np.float32(log_alpha)))
    beta = float(1.0 / (1.0 + math.exp(-float(log_beta))) * 0.1)
    T_steps = x.shape[0]

    import ml_dtypes
    if use_fp8:
        nc = _get_nc(("fp8", T_steps, alpha, beta), T_steps, alpha, beta,
                     lp_dtype=FP8, wh_scale=4096.0)
        in_maps = prepare_in_maps(x, h0, W, W_h, b, alpha, beta,
                                  lp_np=ml_dtypes.float8_e4m3, wh_scale=4096.0)
    else:
        nc = _get_nc(("v3", T_steps, alpha, beta), build_nc2, T_steps, alpha, beta)
        in_maps = prepare_in_maps2(x, h0, W, W_h, b, alpha, beta)
    res = run_bass_kernel_spmd(nc, in_maps, list(range(NC)), trace=trace)
    outs, h = assemble(res.results, T_steps)
    return outs, h, res


def kernel(x, h0, W, W_h, b, log_alpha, log_beta):
    outs, h, _ = run(x, h0, W, W_h, b, log_alpha, log_beta)
    return outs, h
